# revision 19
# baseline (speedup 1.0000x reference)
"""DMI loss kernel for Trainium2 (8 NeuronCores, data-parallel over batch).

reference:
    preds  = [x, 1-x]  [b, 2, hw]
    labels = [y, 1-y]  [b, 2, hw]
    mat    = preds @ labels.T          (per-sample 2x2)
    loss   = mean(-log(|det(mat)| + 1e-3))

Per sample only three reductions over hw are needed:
    S_x = sum(x), S_y = sum(y), S_xy = sum(x*y)
since det(mat) == hw*S_xy - S_x*S_y (exact algebraic identity).

Sharding: batch 64 -> 8 cores x 8 samples. Each core reduces its 8 samples to
per-partition partial sums on-device; the det/log/mean epilogue runs on host
in float64.

Device pipeline per core (memory-bound, ~404 GB/s/core measured):
  DMA : one 1 MiB HWDGE transfer per tensor per sample, all on the sync-engine
        ring so completion order matches sample order (x0,y0,x1,y1,...).
        The LAST sample is split into column chunks so the end-of-stream
        compute tail is one chunk, not a whole sample.
  DVE : tensor_mul + tensor_reduce (free axis)  -> S_xy per partition
  ACT : activation(Copy, accum_out)             -> S_x, S_y per partition
  out : stats tiles [128, NCOL] DMA'd back; host sums partitions in fp64.
"""

import sys

for _p in ("/opt/trn_rl_repo",):
    if _p not in sys.path:
        sys.path.append(_p)

import numpy as np
from contextlib import ExitStack

import concourse.bass as bass
import concourse.tile as tile
from concourse import bacc, mybir
from concourse.bass_utils import run_bass_kernel_spmd

N_CORES = 8
B = 64
H = W = 512
HW = H * W
S = B // N_CORES      # samples per core
P = 128               # SBUF partitions
F = HW // P           # free dim per partition
TAIL_CHUNKS = 2       # last sample split into chunks to shrink the tail
NCOL = S - 1 + TAIL_CHUNKS
DET_EPS = 0.001

_NC_CACHE = None


def build_nc(reps=1, tail_chunks=TAIL_CHUNKS):
    """reps>1 repeats the full memory pass for slope benchmarking."""
    nc = bacc.Bacc()
    f32 = mybir.dt.float32
    ncol = S - 1 + tail_chunks

    x_ext = nc.declare_dram_parameter("x", [S, P, F], f32, isOutput=False)
    y_ext = nc.declare_dram_parameter("y", [S, P, F], f32, isOutput=False)
    o_ext = nc.declare_dram_parameter("o", [3, P, ncol], f32, isOutput=True)

    with tile.TileContext(nc) as tc, ExitStack() as ctx:
        xp = ctx.enter_context(tc.tile_pool(name="x", bufs=S))
        yp = ctx.enter_context(tc.tile_pool(name="y", bufs=S))
        scr = ctx.enter_context(tc.tile_pool(name="scr", bufs=1))
        stat = ctx.enter_context(tc.tile_pool(name="stat", bufs=1))

        sx = stat.tile([P, ncol], f32, tag="sx")
        sy = stat.tile([P, ncol], f32, tag="sy")
        sxy = stat.tile([P, ncol], f32, tag="sxy")

        dve_scr = scr.tile([P, F], f32, tag="dve_scr")
        act_scr = scr.tile([P, F], f32, tag="act_scr")

        for rep in range(reps):
            for s in range(S):
                nchunk = tail_chunks if s == S - 1 else 1
                fc = F // nchunk
                xt = xp.tile([P, F], f32, tag="xt", name=f"xt{rep}_{s}")
                yt = yp.tile([P, F], f32, tag="yt", name=f"yt{rep}_{s}")
                for c in range(nchunk):
                    cs = slice(c * fc, (c + 1) * fc)
                    col = s + c
                    nc.sync.dma_start(xt[:, cs], x_ext[s, :, cs])
                    nc.sync.dma_start(yt[:, cs], y_ext[s, :, cs])
                    # S_xy: multiply then free-axis reduce on DVE
                    nc.vector.tensor_mul(dve_scr[:, cs], xt[:, cs], yt[:, cs])
                    nc.vector.tensor_reduce(
                        out=sxy[:, col : col + 1],
                        in_=dve_scr[:, cs],
                        axis=mybir.AxisListType.X,
                        op=mybir.AluOpType.add,
                    )
                    # S_x, S_y: copy-with-accumulate on ACT
                    nc.scalar.activation(
                        out=act_scr[:, cs],
                        in_=xt[:, cs],
                        func=mybir.ActivationFunctionType.Copy,
                        accum_out=sx[:, col : col + 1],
                    )
                    nc.scalar.activation(
                        out=act_scr[:, cs],
                        in_=yt[:, cs],
                        func=mybir.ActivationFunctionType.Copy,
                        accum_out=sy[:, col : col + 1],
                    )

        # Ship per-partition stats; host does the 128-partition sum in fp64.
        nc.sync.dma_start(o_ext[0], sx[:])
        nc.sync.dma_start(o_ext[1], sy[:])
        nc.sync.dma_start(o_ext[2], sxy[:])

    nc.compile()
    return nc


def _get_nc():
    global _NC_CACHE
    if _NC_CACHE is None:
        _NC_CACHE = build_nc()
    return _NC_CACHE


def _device_sums(input, target, trace=False, **kw):
    """Run the Bass kernel; return (sx, sy, sxy) each [B] float64, plus results."""
    x = np.ascontiguousarray(np.asarray(input, dtype=np.float32)).reshape(
        N_CORES, S, P, F
    )
    y = np.ascontiguousarray(np.asarray(target, dtype=np.float32)).reshape(
        N_CORES, S, P, F
    )
    nc = _get_nc()
    in_maps = [{"x": x[c], "y": y[c]} for c in range(N_CORES)]
    res = run_bass_kernel_spmd(nc, in_maps, list(range(N_CORES)), trace=trace, **kw)
    sx = np.empty(B, np.float64)
    sy = np.empty(B, np.float64)
    sxy = np.empty(B, np.float64)

    def unpack(cols):
        # cols [NCOL] (already partition-summed): samples 0..S-2 one column
        # each; the last sample is the sum of the TAIL_CHUNKS trailing columns
        out = np.empty(S, np.float64)
        out[: S - 1] = cols[: S - 1]
        out[S - 1] = cols[S - 1 :].sum()
        return out

    for c in range(N_CORES):
        o = np.asarray(res.results[c]["o"], np.float64)  # [3, P, NCOL]
        sums = o.sum(axis=1)  # [3, NCOL], fp64 partition reduction
        sx[c * S : (c + 1) * S] = unpack(sums[0])
        sy[c * S : (c + 1) * S] = unpack(sums[1])
        sxy[c * S : (c + 1) * S] = unpack(sums[2])
    return sx, sy, sxy, res


def _loss_from_sums(sx, sy, sxy):
    # mat = [[S_xy, S_x-S_xy], [S_y-S_xy, HW-S_x-S_y+S_xy]]; det = HW*S_xy - S_x*S_y
    m00 = sxy
    m01 = sx - sxy
    m10 = sy - sxy
    m11 = HW - sx - sy + sxy
    det = m00 * m11 - m01 * m10
    loss = -np.log(np.abs(det) + DET_EPS)
    return np.array(loss.mean(), dtype=np.float32)


def kernel(input, target):
    sx, sy, sxy, _ = _device_sums(input, target)
    return _loss_from_sums(sx, sy, sxy)


if __name__ == "__main__":
    rng = np.random.default_rng(0)
    x = rng.random((B, 1, H, W), dtype=np.float32)
    y = rng.random((B, 1, H, W), dtype=np.float32)
    got = kernel(input=x, target=y)
    xf = x.reshape(B, -1).astype(np.float64)
    yf = y.reshape(B, -1).astype(np.float64)
    det = HW * (xf * yf).sum(1) - xf.sum(1) * yf.sum(1)
    want = (-np.log(np.abs(det) + DET_EPS)).mean()
    print("kernel:", got, "numpy:", want, "rel:", abs(got - want) / abs(want))


# revision 26
# speedup vs baseline: 1.2549x; 1.2549x over previous
"""DMI loss kernel for Trainium2 (8 NeuronCores, data-parallel over batch).

reference:
    preds  = [x, 1-x]  [b, 2, hw]
    labels = [y, 1-y]  [b, 2, hw]
    mat    = preds @ labels.T          (per-sample 2x2)
    loss   = mean(-log(|det(mat)| + 1e-3))

Per sample only three reductions over hw are needed:
    S_x = sum(x), S_y = sum(y), S_xy = sum(x*y)
since det(mat) == hw*S_xy - S_x*S_y (exact algebraic identity).

Sharding: batch 64 -> 8 cores x 8 samples. Each core reduces its 8 samples to
per-partition partial sums on-device; the det/log/mean epilogue runs on host
in float64.

Device pipeline per core (memory-bound, ~404 GB/s/core measured):
  DMA : one 1 MiB HWDGE transfer per tensor per sample, all on the sync-engine
        ring so completion order matches sample order (x0,y0,x1,y1,...).
        The LAST sample is split into column chunks so the end-of-stream
        compute tail is one chunk, not a whole sample.
  DVE : tensor_mul + tensor_reduce (free axis)  -> S_xy per partition
  ACT : activation(Copy, accum_out)             -> S_x, S_y per partition
  out : stats tiles [128, NCOL] DMA'd back; host sums partitions in fp64.
"""

import sys

for _p in ("/opt/trn_rl_repo",):
    if _p not in sys.path:
        sys.path.append(_p)

import numpy as np
from contextlib import ExitStack

import concourse.bass as bass
import concourse.tile as tile
from concourse import bacc, mybir
from concourse.bass_utils import run_bass_kernel_spmd

N_CORES = 8
B = 64
H = W = 512
HW = H * W
S = B // N_CORES      # samples per core
P = 128               # SBUF partitions
F = HW // P           # free dim per partition
TAIL_CHUNKS = 2       # last sample split into chunks to shrink the tail
N_PAIRS = 0           # 2MiB sample-pair transfers disabled: the cost-model
                      # shows they delay pipeline ramp more than they save
N_SINGLE = S - 2 * N_PAIRS
NCOL = N_PAIRS + (N_SINGLE - 1) + TAIL_CHUNKS
DET_EPS = 0.001

_NC_CACHE = None


def build_nc(reps=1, tail_chunks=TAIL_CHUNKS, pairs=N_PAIRS):
    """reps>1 repeats the full memory pass for slope benchmarking.

    The first 2*pairs samples are streamed as 2 MiB sample-PAIR transfers: a
    contiguous pair block viewed as [128, 4096] puts sample 2q in partitions
    0..63 and sample 2q+1 in 64..127, so one DMA + one op chain covers two
    samples; the host separates them by partition range. Remaining samples
    stream as 1 MiB singles, with the last sample column-chunked to shrink
    the end-of-stream tail.
    """
    nc = bacc.Bacc()
    f32 = mybir.dt.float32
    n_single = S - 2 * pairs
    ncol = pairs + (n_single - 1) + tail_chunks

    x_ext = nc.declare_dram_parameter("x", [S, P, F], f32, isOutput=False)
    y_ext = nc.declare_dram_parameter("y", [S, P, F], f32, isOutput=False)
    o_ext = nc.declare_dram_parameter("o", [3, P, ncol], f32, isOutput=True)
    # contiguous pair view of the same buffers: [S//2, 128, 2F], where pair q
    # holds sample 2q in partitions 0..63 and sample 2q+1 in 64..127
    x_pair = x_ext.rearrange("(q a) (p2 b) f -> q (a p2) (b f)", a=2, b=2)
    y_pair = y_ext.rearrange("(q a) (p2 b) f -> q (a p2) (b f)", a=2, b=2)

    with tile.TileContext(nc) as tc, ExitStack() as ctx:
        xqp = ctx.enter_context(tc.tile_pool(name="xq", bufs=max(pairs, 1)))
        yqp = ctx.enter_context(tc.tile_pool(name="yq", bufs=max(pairs, 1)))
        xp = ctx.enter_context(tc.tile_pool(name="x", bufs=max(n_single, 1)))
        yp = ctx.enter_context(tc.tile_pool(name="y", bufs=max(n_single, 1)))
        scr = ctx.enter_context(tc.tile_pool(name="scr", bufs=1))
        stat = ctx.enter_context(tc.tile_pool(name="stat", bufs=1))

        sx = stat.tile([P, ncol], f32, tag="sx")
        sy = stat.tile([P, ncol], f32, tag="sy")
        sxy = stat.tile([P, ncol], f32, tag="sxy")

        dve_scr = scr.tile([P, 2 * F], f32, tag="dve_scr")
        act_scr = scr.tile([P, 2 * F], f32, tag="act_scr")

        def chain(col, xt_ap, yt_ap, ds, as_):
            nc.vector.tensor_mul(ds, xt_ap, yt_ap)
            nc.vector.tensor_reduce(
                out=sxy[:, col : col + 1],
                in_=ds,
                axis=mybir.AxisListType.X,
                op=mybir.AluOpType.add,
            )
            nc.scalar.activation(
                out=as_,
                in_=xt_ap,
                func=mybir.ActivationFunctionType.Copy,
                accum_out=sx[:, col : col + 1],
            )
            nc.scalar.activation(
                out=as_,
                in_=yt_ap,
                func=mybir.ActivationFunctionType.Copy,
                accum_out=sy[:, col : col + 1],
            )

        for rep in range(reps):
            for q in range(pairs):
                xt = xqp.tile([P, 2 * F], f32, tag="xq", name=f"xq{rep}_{q}")
                yt = yqp.tile([P, 2 * F], f32, tag="yq", name=f"yq{rep}_{q}")
                nc.sync.dma_start(xt[:], x_pair[q])
                nc.sync.dma_start(yt[:], y_pair[q])
                chain(q, xt[:], yt[:], dve_scr[:], act_scr[:])

            for i, s in enumerate(range(2 * pairs, S)):
                nchunk = tail_chunks if s == S - 1 else 1
                fc = F // nchunk
                xt = xp.tile([P, F], f32, tag="xt", name=f"xt{rep}_{s}")
                yt = yp.tile([P, F], f32, tag="yt", name=f"yt{rep}_{s}")
                for c in range(nchunk):
                    cs = slice(c * fc, (c + 1) * fc)
                    col = pairs + i + c
                    nc.sync.dma_start(xt[:, cs], x_ext[s, :, cs])
                    nc.sync.dma_start(yt[:, cs], y_ext[s, :, cs])
                    chain(col, xt[:, cs], yt[:, cs], dve_scr[:, cs], act_scr[:, cs])

        # Ship per-partition stats; host does the 128-partition sum in fp64.
        nc.sync.dma_start(o_ext[0], sx[:])
        nc.sync.dma_start(o_ext[1], sy[:])
        nc.sync.dma_start(o_ext[2], sxy[:])

    nc.compile()
    return nc


def _get_nc():
    global _NC_CACHE
    if _NC_CACHE is None:
        _NC_CACHE = build_nc()
    return _NC_CACHE


def _device_sums(input, target, trace=False, **kw):
    """Run the Bass kernel; return (sx, sy, sxy) each [B] float64, plus results."""
    x = np.ascontiguousarray(np.asarray(input, dtype=np.float32)).reshape(
        N_CORES, S, P, F
    )
    y = np.ascontiguousarray(np.asarray(target, dtype=np.float32)).reshape(
        N_CORES, S, P, F
    )
    nc = _get_nc()
    in_maps = [{"x": x[c], "y": y[c]} for c in range(N_CORES)]
    res = run_bass_kernel_spmd(nc, in_maps, list(range(N_CORES)), trace=trace, **kw)
    sx = np.empty(B, np.float64)
    sy = np.empty(B, np.float64)
    sxy = np.empty(B, np.float64)

    def unpack(o_t):
        # o_t [P, NCOL] per-partition stats, fp64 partition reduction on host.
        # cols 0..N_PAIRS-1: pair q -> sample 2q in partitions 0:64,
        #   sample 2q+1 in partitions 64:128
        # cols N_PAIRS..: singles, last sample = sum of TAIL_CHUNKS tail cols
        out = np.empty(S, np.float64)
        for q in range(N_PAIRS):
            out[2 * q] = o_t[: P // 2, q].sum()
            out[2 * q + 1] = o_t[P // 2 :, q].sum()
        full = o_t.sum(axis=0)  # [NCOL]
        for i in range(N_SINGLE - 1):
            out[2 * N_PAIRS + i] = full[N_PAIRS + i]
        out[S - 1] = full[N_PAIRS + N_SINGLE - 1 :].sum()
        return out

    for c in range(N_CORES):
        o = np.asarray(res.results[c]["o"], np.float64)  # [3, P, NCOL]
        sx[c * S : (c + 1) * S] = unpack(o[0])
        sy[c * S : (c + 1) * S] = unpack(o[1])
        sxy[c * S : (c + 1) * S] = unpack(o[2])
    return sx, sy, sxy, res


def _loss_from_sums(sx, sy, sxy):
    # mat = [[S_xy, S_x-S_xy], [S_y-S_xy, HW-S_x-S_y+S_xy]]; det = HW*S_xy - S_x*S_y
    m00 = sxy
    m01 = sx - sxy
    m10 = sy - sxy
    m11 = HW - sx - sy + sxy
    det = m00 * m11 - m01 * m10
    loss = -np.log(np.abs(det) + DET_EPS)
    return np.array(loss.mean(), dtype=np.float32)


def kernel(input, target):
    sx, sy, sxy, _ = _device_sums(input, target)
    return _loss_from_sums(sx, sy, sxy)


if __name__ == "__main__":
    rng = np.random.default_rng(0)
    x = rng.random((B, 1, H, W), dtype=np.float32)
    y = rng.random((B, 1, H, W), dtype=np.float32)
    got = kernel(input=x, target=y)
    xf = x.reshape(B, -1).astype(np.float64)
    yf = y.reshape(B, -1).astype(np.float64)
    det = HW * (xf * yf).sum(1) - xf.sum(1) * yf.sum(1)
    want = (-np.log(np.abs(det) + DET_EPS)).mean()
    print("kernel:", got, "numpy:", want, "rel:", abs(got - want) / abs(want))


# revision 29
# speedup vs baseline: 1.2566x; 1.0013x over previous
"""DMI loss kernel for Trainium2 (8 NeuronCores, data-parallel over batch).

reference:
    preds  = [x, 1-x]  [b, 2, hw]
    labels = [y, 1-y]  [b, 2, hw]
    mat    = preds @ labels.T          (per-sample 2x2)
    loss   = mean(-log(|det(mat)| + 1e-3))

Per sample only three reductions over hw are needed:
    S_x = sum(x), S_y = sum(y), S_xy = sum(x*y)
since det(mat) == hw*S_xy - S_x*S_y (exact algebraic identity).

Sharding: batch 64 -> 8 cores x 8 samples. Each core reduces its 8 samples to
per-partition partial sums on-device; the det/log/mean epilogue runs on host
in float64.

Device pipeline per core (memory-bound, ~404 GB/s/core measured):
  DMA : one 1 MiB HWDGE transfer per tensor per sample, all on the sync-engine
        ring so completion order matches sample order (x0,y0,x1,y1,...).
        The LAST sample is split into column chunks so the end-of-stream
        compute tail is one chunk, not a whole sample.
  DVE : tensor_mul + tensor_reduce (free axis)  -> S_xy per partition
  ACT : activation(Copy, accum_out)             -> S_x, S_y per partition
  out : stats tiles [128, NCOL] DMA'd back; host sums partitions in fp64.
"""

import sys

for _p in ("/opt/trn_rl_repo",):
    if _p not in sys.path:
        sys.path.append(_p)

import numpy as np
from contextlib import ExitStack

import concourse.bass as bass
import concourse.tile as tile
from concourse import bacc, mybir
from concourse.bass_utils import run_bass_kernel_spmd

N_CORES = 8
B = 64
H = W = 512
HW = H * W
S = B // N_CORES      # samples per core
P = 128               # SBUF partitions
F = HW // P           # free dim per partition
TAIL_SPLITS = (1024, 1536, 2048)  # uneven chunk boundaries for the last
                                  # sample: the final chunks are small so the
                                  # end-of-stream DVE tail is short
TAIL_CHUNKS = len(TAIL_SPLITS)
N_PAIRS = 0           # 2MiB sample-pair transfers disabled: the cost-model
                      # shows they delay pipeline ramp more than they save
N_SINGLE = S - 2 * N_PAIRS
NCOL = N_PAIRS + (N_SINGLE - 1) + TAIL_CHUNKS
DET_EPS = 0.001

_NC_CACHE = None


def build_nc(reps=1, tail_chunks=TAIL_CHUNKS, pairs=N_PAIRS):
    """reps>1 repeats the full memory pass for slope benchmarking.

    The first 2*pairs samples are streamed as 2 MiB sample-PAIR transfers: a
    contiguous pair block viewed as [128, 4096] puts sample 2q in partitions
    0..63 and sample 2q+1 in 64..127, so one DMA + one op chain covers two
    samples; the host separates them by partition range. Remaining samples
    stream as 1 MiB singles, with the last sample column-chunked to shrink
    the end-of-stream tail.
    """
    nc = bacc.Bacc()
    f32 = mybir.dt.float32
    n_single = S - 2 * pairs
    ncol = pairs + (n_single - 1) + tail_chunks

    x_ext = nc.declare_dram_parameter("x", [S, P, F], f32, isOutput=False)
    y_ext = nc.declare_dram_parameter("y", [S, P, F], f32, isOutput=False)
    o_ext = nc.declare_dram_parameter("o", [3, P, ncol], f32, isOutput=True)
    # contiguous pair view of the same buffers: [S//2, 128, 2F], where pair q
    # holds sample 2q in partitions 0..63 and sample 2q+1 in 64..127
    x_pair = x_ext.rearrange("(q a) (p2 b) f -> q (a p2) (b f)", a=2, b=2)
    y_pair = y_ext.rearrange("(q a) (p2 b) f -> q (a p2) (b f)", a=2, b=2)

    with tile.TileContext(nc) as tc, ExitStack() as ctx:
        xqp = ctx.enter_context(tc.tile_pool(name="xq", bufs=max(pairs, 1)))
        yqp = ctx.enter_context(tc.tile_pool(name="yq", bufs=max(pairs, 1)))
        xp = ctx.enter_context(tc.tile_pool(name="x", bufs=max(n_single, 1)))
        yp = ctx.enter_context(tc.tile_pool(name="y", bufs=max(n_single, 1)))
        scr = ctx.enter_context(tc.tile_pool(name="scr", bufs=1))
        stat = ctx.enter_context(tc.tile_pool(name="stat", bufs=1))

        sx = stat.tile([P, ncol], f32, tag="sx")
        sy = stat.tile([P, ncol], f32, tag="sy")
        sxy = stat.tile([P, ncol], f32, tag="sxy")

        dve_scr = scr.tile([P, 2 * F], f32, tag="dve_scr")
        act_scr = scr.tile([P, 2 * F], f32, tag="act_scr")

        def chain(col, xt_ap, yt_ap, ds, as_):
            nc.vector.tensor_mul(ds, xt_ap, yt_ap)
            nc.vector.tensor_reduce(
                out=sxy[:, col : col + 1],
                in_=ds,
                axis=mybir.AxisListType.X,
                op=mybir.AluOpType.add,
            )
            nc.scalar.activation(
                out=as_,
                in_=xt_ap,
                func=mybir.ActivationFunctionType.Copy,
                accum_out=sx[:, col : col + 1],
            )
            nc.scalar.activation(
                out=as_,
                in_=yt_ap,
                func=mybir.ActivationFunctionType.Copy,
                accum_out=sy[:, col : col + 1],
            )

        for rep in range(reps):
            for q in range(pairs):
                xt = xqp.tile([P, 2 * F], f32, tag="xq", name=f"xq{rep}_{q}")
                yt = yqp.tile([P, 2 * F], f32, tag="yq", name=f"yq{rep}_{q}")
                nc.sync.dma_start(xt[:], x_pair[q])
                nc.sync.dma_start(yt[:], y_pair[q])
                chain(q, xt[:], yt[:], dve_scr[:], act_scr[:])

            tail_bounds = (
                TAIL_SPLITS
                if tail_chunks == TAIL_CHUNKS
                else tuple(F // tail_chunks * (k + 1) for k in range(tail_chunks))
            )
            for i, s in enumerate(range(2 * pairs, S)):
                bounds = tail_bounds if s == S - 1 else (F,)
                xt = xp.tile([P, F], f32, tag="xt", name=f"xt{rep}_{s}")
                yt = yp.tile([P, F], f32, tag="yt", name=f"yt{rep}_{s}")
                lo = 0
                for c, hi in enumerate(bounds):
                    cs = slice(lo, hi)
                    lo = hi
                    col = pairs + i + c
                    nc.sync.dma_start(xt[:, cs], x_ext[s, :, cs])
                    nc.sync.dma_start(yt[:, cs], y_ext[s, :, cs])
                    chain(col, xt[:, cs], yt[:, cs], dve_scr[:, cs], act_scr[:, cs])

        # Ship per-partition stats; host does the 128-partition sum in fp64.
        nc.sync.dma_start(o_ext[0], sx[:])
        nc.sync.dma_start(o_ext[1], sy[:])
        nc.sync.dma_start(o_ext[2], sxy[:])

    nc.compile()
    return nc


def _get_nc():
    global _NC_CACHE
    if _NC_CACHE is None:
        _NC_CACHE = build_nc()
    return _NC_CACHE


def _device_sums(input, target, trace=False, **kw):
    """Run the Bass kernel; return (sx, sy, sxy) each [B] float64, plus results."""
    x = np.ascontiguousarray(np.asarray(input, dtype=np.float32)).reshape(
        N_CORES, S, P, F
    )
    y = np.ascontiguousarray(np.asarray(target, dtype=np.float32)).reshape(
        N_CORES, S, P, F
    )
    nc = _get_nc()
    in_maps = [{"x": x[c], "y": y[c]} for c in range(N_CORES)]
    res = run_bass_kernel_spmd(nc, in_maps, list(range(N_CORES)), trace=trace, **kw)
    sx = np.empty(B, np.float64)
    sy = np.empty(B, np.float64)
    sxy = np.empty(B, np.float64)

    def unpack(o_t):
        # o_t [P, NCOL] per-partition stats, fp64 partition reduction on host.
        # cols 0..N_PAIRS-1: pair q -> sample 2q in partitions 0:64,
        #   sample 2q+1 in partitions 64:128
        # cols N_PAIRS..: singles, last sample = sum of TAIL_CHUNKS tail cols
        out = np.empty(S, np.float64)
        for q in range(N_PAIRS):
            out[2 * q] = o_t[: P // 2, q].sum()
            out[2 * q + 1] = o_t[P // 2 :, q].sum()
        full = o_t.sum(axis=0)  # [NCOL]
        for i in range(N_SINGLE - 1):
            out[2 * N_PAIRS + i] = full[N_PAIRS + i]
        out[S - 1] = full[N_PAIRS + N_SINGLE - 1 :].sum()
        return out

    for c in range(N_CORES):
        o = np.asarray(res.results[c]["o"], np.float64)  # [3, P, NCOL]
        sx[c * S : (c + 1) * S] = unpack(o[0])
        sy[c * S : (c + 1) * S] = unpack(o[1])
        sxy[c * S : (c + 1) * S] = unpack(o[2])
    return sx, sy, sxy, res


def _loss_from_sums(sx, sy, sxy):
    # mat = [[S_xy, S_x-S_xy], [S_y-S_xy, HW-S_x-S_y+S_xy]]; det = HW*S_xy - S_x*S_y
    m00 = sxy
    m01 = sx - sxy
    m10 = sy - sxy
    m11 = HW - sx - sy + sxy
    det = m00 * m11 - m01 * m10
    loss = -np.log(np.abs(det) + DET_EPS)
    return np.array(loss.mean(), dtype=np.float32)


def kernel(input, target):
    sx, sy, sxy, _ = _device_sums(input, target)
    return _loss_from_sums(sx, sy, sxy)


if __name__ == "__main__":
    rng = np.random.default_rng(0)
    x = rng.random((B, 1, H, W), dtype=np.float32)
    y = rng.random((B, 1, H, W), dtype=np.float32)
    got = kernel(input=x, target=y)
    xf = x.reshape(B, -1).astype(np.float64)
    yf = y.reshape(B, -1).astype(np.float64)
    det = HW * (xf * yf).sum(1) - xf.sum(1) * yf.sum(1)
    want = (-np.log(np.abs(det) + DET_EPS)).mean()
    print("kernel:", got, "numpy:", want, "rel:", abs(got - want) / abs(want))
